# revision 1
# baseline (speedup 1.0000x reference)
"""Differential attention on 8 trn2 NeuronCores.

Sharding: data-parallel over batch (2 groups of 4 cores) x tensor-parallel
over heads (4 heads/core). Each core computes its head-group's qkv
projections, dual softmax attention, and a partial output projection over
its 256 channels, plus the per-token sum-of-squares needed for the RMSNorm.
The host sums the 4 partial projections per batch, applies the RMS scale
(which commutes with the channel contraction) and the bias.

All matmuls run as float32r (full-rate fp32 streaming on the PE).
Layouts are feature-major ([feature, token]) so softmax rowsums ride the
attention@V matmul via a ones-augmented V, avoiding cross-partition
reductions.

Phase B runs query-chunk-major so each 512-token projection chunk is
emitted as soon as its four heads' combines land, overlapping the output
projection and DMA with the remaining attention work. The score->exp->
attn@V chain is software-pipelined two key-tiles deep; exp runs on the
Act engine while the combine (fast NR reciprocal + broadcast + muls) is
split across DVE and Pool so no engine sits behind another's queue.
"""
import sys

sys.path.insert(0, "/opt/trn_rl_repo")

import numpy as np

import concourse.bass as bass
import concourse.mybir as mybir
import concourse.tile as tile
from concourse import bacc, bass_utils
from concourse.bass_interp import get_hw_module

F32 = mybir.dt.float32
F32R = mybir.dt.float32r
BF16 = mybir.dt.bfloat16
AF = mybir.ActivationFunctionType
OP = mybir.AluOpType
AX = mybir.AxisListType

B, N, DIM = 2, 2048, 1024
H, HD = 16, 64
HPC = 4          # heads per core
CH = HPC * HD    # channels per core (256)
SCALE = HD ** -0.5
EPS = 1e-5
NT = N // 128    # 16 token tiles
QC = N // 512    # 4 query chunks
CT = DIM // 128  # 8 contraction tiles


def r(ap):
    return ap.bitcast(F32R)


def build_program(nc):
    xt = nc.dram_tensor("xt", [DIM, N], F32, kind="ExternalInput").ap()
    wqk = nc.dram_tensor("wqk", [128, CT * 8 * 128], F32, kind="ExternalInput").ap()
    wv = nc.dram_tensor("wv", [128, CT * CH], F32, kind="ExternalInput").ap()
    wp = nc.dram_tensor("wp", [CH, DIM], F32, kind="ExternalInput").ap()
    lam = nc.dram_tensor("lam", [1, 4 * HD], F32, kind="ExternalInput").ap()
    out = nc.dram_tensor("out", [DIM, N], F32, kind="ExternalOutput").ap()
    ssq = nc.dram_tensor("ssq", [1, N], F32, kind="ExternalOutput").ap()

    with tile.TileContext(nc) as tc:
        with (
            nc.allow_low_precision(reason="float32r matmul operand rounding is intentional"),
            tc.tile_pool(name="persist", bufs=1) as pp,
            tc.tile_pool(name="qkp", bufs=8) as qkpool,
            tc.tile_pool(name="opool", bufs=2) as opool,
        ):
            # ---- constants / lambda ----
            ones128 = pp.tile([128, 1], F32R, tag="ones128")
            nc.vector.memset(ones128.bitcast(mybir.dt.uint32)[:], 0x3F800000)
            lam_sb = pp.tile([1, 4 * HD], F32, tag="lam")
            nc.sync.dma_start(lam_sb[:], lam[:])
            lprod = pp.tile([1, 2 * HD], F32, tag="lprod")
            nc.vector.tensor_mul(lprod[:, 0:HD], lam_sb[:, 0:HD], lam_sb[:, HD:2 * HD])
            nc.vector.tensor_mul(
                lprod[:, HD:2 * HD], lam_sb[:, 2 * HD:3 * HD], lam_sb[:, 3 * HD:4 * HD]
            )
            lsum = pp.tile([1, 2], F32, tag="lsum")
            nc.vector.reduce_sum(lsum[:, 0:1], lprod[:, 0:HD], axis=AX.X)
            nc.vector.reduce_sum(lsum[:, 1:2], lprod[:, HD:2 * HD], axis=AX.X)
            lexp = pp.tile([1, 2], F32, tag="lexp")
            nc.scalar.activation(lexp[:], lsum[:], AF.Exp)
            negl = pp.tile([1, 1], F32, tag="negl")
            # -lambda_full = exp(sum lq2*lk2) - exp(sum lq1*lk1) - 0.8
            nc.vector.tensor_sub(negl[:], lexp[:, 1:2], lexp[:, 0:1])
            nc.vector.tensor_scalar_add(negl[:], negl[:], -0.8)

            # ---- persistent big tiles ----
            # v augmented with a ones column: [token_part, head, token_tile, hd+1]
            vaug = pp.tile([128, HPC, NT, HD + 1], F32R, tag="vaug")
            nc.vector.memset(vaug[:, :, :, HD:HD + 1].bitcast(mybir.dt.uint32), 0x3F800000)
            # q/k tiles in bf16: halves SBUF and makes the PE weight loads
            # single-pass; scores matmuls run bf16 x bf16 at the same
            # 1 cycle/row streaming rate as fp32r
            qk = [qkpool.tile([128, N], BF16, tag="qk", name=f"qk{i}") for i in range(8)]
            wp_sb = pp.tile([128, 2, DIM], F32R, tag="wp")
            nc.sync.dma_start(wp_sb[:], wp.rearrange("(t p) o -> p t o", p=128).bitcast(F32R))
            o_t = [opool.tile([128, N], F32R, tag="obig", name=f"obig{i}") for i in range(2)]
            ssq_sb = pp.tile([1, N], F32, tag="ssqs")

            # ---- phase A: projections (V and QK interleaved by x chunk) ----
            with (
                tc.tile_pool(name="xa", bufs=1) as xpool,
                tc.tile_pool(name="wva", bufs=1) as wvpool,
                tc.tile_pool(name="wqka", bufs=1) as wqkpool,
                tc.tile_pool(name="psA", bufs=3, space="PSUM") as psA,
                tc.tile_pool(name="psV", bufs=2, space="PSUM") as psV,
            ):
                wv_sb = wvpool.tile([128, CT, CH], F32R, tag="wv")
                wv_flat = wv_sb.rearrange("p t f -> p (t f)")
                for wh in range(2):
                    nc.sync.dma_start(
                        wv_flat[:, wh * 1024:(wh + 1) * 1024],
                        wv[:, wh * 1024:(wh + 1) * 1024].bitcast(F32R),
                    )
                x_sb = xpool.tile([128, CT, N], F32R, tag="x")
                xt_r = xt.rearrange("(t p) n -> p t n", p=128)
                wqk_sb = wqkpool.tile([128, CT, 8 * 128], F32R, tag="wqk")
                wqk_flat = wqk_sb.rearrange("p t f -> p (t f)")
                # small first x chunk so V(0) starts early, then the qk
                # weights, then the remaining x chunks
                for lo, hi in [(0, 128), (128, 512)]:
                    nc.sync.dma_start(
                        x_sb[:, :, lo:hi], xt_r[:, :, lo:hi].bitcast(F32R)
                    )
                for wc in range(4):
                    nc.sync.dma_start(
                        wqk_flat[:, wc * 2048:(wc + 1) * 2048],
                        wqk[:, wc * 2048:(wc + 1) * 2048].bitcast(F32R),
                    )
                for xc in range(6):
                    lo = 512 + xc * 256
                    nc.sync.dma_start(
                        x_sb[:, :, lo:lo + 256],
                        xt_r[:, :, lo:lo + 256].bitcast(F32R),
                    )

                def emit_v(nt):
                    # V in [token, feature] layout, scattered into vaug
                    ps = psV.tile([128, CH], F32, tag="psv")
                    for ct in range(CT):
                        nc.tensor.matmul(
                            ps[:],
                            lhsT=r(x_sb[:, ct, nt * 128:(nt + 1) * 128]),
                            rhs=r(wv_sb[:, ct, :]),
                            start=(ct == 0),
                            stop=(ct == CT - 1),
                        )
                    nc.scalar.copy(
                        out=vaug[:, :, nt, 0:HD],
                        in_=ps.rearrange("p (h d) -> p h d", d=HD),
                    )

                def emit_qk(ft, qc):
                    # Q/K in [feature, token] layout.
                    # f-tile ft<4: [q1(h=ft) 64 | q2(h=ft) 64]; ft>=4: [k1|k2] of h=ft-4
                    ps = psA.tile([128, 512], F32, tag="psqk")
                    for ct in range(CT):
                        nc.tensor.matmul(
                            ps[:],
                            lhsT=r(wqk_sb[:, ct, ft * 128:(ft + 1) * 128]),
                            rhs=r(x_sb[:, ct, qc * 512:(qc + 1) * 512]),
                            start=(ct == 0),
                            stop=(ct == CT - 1),
                        )
                    nc.vector.tensor_copy(qk[ft][:, qc * 512:(qc + 1) * 512], ps[:])

                for xc in range(4):
                    for nt in range(xc * 4, xc * 4 + 4):
                        emit_v(nt)
                    for ft in range(8):
                        emit_qk(ft, xc)

            # ---- phase B: attention, combine, and chunked projection ----
            with tc.tile_pool(name="sqp", bufs=1) as sqpool:
              sq = [sqpool.tile([128, N], F32R, tag=f"sq{i}", name=f"sq{i}")
                    for i in range(2)]
              with (
                tc.tile_pool(name="slots", bufs=3, space="PSUM") as slots,
                tc.tile_pool(name="po", bufs=2, space="PSUM") as po,
                tc.tile_pool(name="upool", bufs=8) as upool,
                tc.tile_pool(name="cpool", bufs=2) as cpool,
                tc.tile_pool(name="rpool", bufs=2) as rpool,
                tc.tile_pool(name="obuf", bufs=3) as obuf,
              ):
                def emit_proj_pair(qc, pair):
                    # two projection column-tiles of a finished 512-token
                    # chunk; pairs are spread across the next chunk's heads
                    # so each psum evacuation has a whole head of DVE slack
                    for ot in (2 * pair, 2 * pair + 1):
                        ps = po.tile([128, 512], F32, tag="oacc",
                                     name=f"psp_{qc}_{ot}")
                        for t in range(2):
                            nc.tensor.matmul(
                                ps[:],
                                lhsT=r(wp_sb[:, t, ot * 128:(ot + 1) * 128]),
                                rhs=r(o_t[t][:, qc * 512:(qc + 1) * 512]),
                                start=(t == 0),
                                stop=(t == 1),
                            )
                        ob = obuf.tile([128, 512], F32, tag="ob",
                                       name=f"ob_{qc}_{ot}")
                        nc.vector.tensor_copy(ob[:], ps[:])
                        nc.sync.dma_start(
                            out[ot * 128:(ot + 1) * 128, qc * 512:(qc + 1) * 512],
                            ob[:],
                        )
                    if pair == 3:
                        # squares for the RMS sum-of-squares, on DVE
                        for t in range(2):
                            nc.vector.tensor_mul(
                                sq[t][:, qc * 512:(qc + 1) * 512],
                                o_t[t][:, qc * 512:(qc + 1) * 512],
                                o_t[t][:, qc * 512:(qc + 1) * 512],
                            )

                for qc in range(QC):
                    for h in range(HPC):
                        tq, tk = qk[h], qk[4 + h]
                        o1 = po.tile([128, 512], F32, tag="oacc", name=f"o1_{qc}_{h}")
                        o2 = po.tile([128, 512], F32, tag="oacc", name=f"o2_{qc}_{h}")

                        def emit_attnv(kt, u):
                            for j, o in enumerate((o1, o2)):
                                nc.tensor.matmul(
                                    o[0:HD + 1, :],
                                    lhsT=r(vaug[:, h, kt, :]),
                                    rhs=u[:, j * 512:(j + 1) * 512],
                                    start=(kt == 0),
                                    stop=(kt == NT - 1),
                                )

                        pend = []
                        for g in range(NT):
                            sl = slots.tile([128, 1024], F32, tag="slot",
                                            name=f"sl_{qc}_{h}_{g}")
                            for term in range(2):
                                rb = term * 64
                                nc.tensor.matmul(
                                    sl[:, term * 512:(term + 1) * 512],
                                    lhsT=tk[rb:rb + 64, g * 128:(g + 1) * 128],
                                    rhs=tq[rb:rb + 64, qc * 512:(qc + 1) * 512],
                                    start=True,
                                    stop=True,
                                )
                            u = upool.tile([128, 1024], F32R, tag="u",
                                           name=f"u_{qc}_{h}_{g}")
                            nc.scalar.activation(u[:], sl[:], AF.Exp, scale=SCALE)
                            pend.append((g, u))
                            if g >= 2:
                                emit_attnv(*pend.pop(0))
                        for ent in pend:
                            emit_attnv(*ent)
                        pend.clear()

                        # combine: out_h = num1/r1 - lambda*num2/r2.
                        # evacuate psum on DVE -- numerator rows into o12,
                        # the rowsum rows into partition 0 of rsum (the
                        # custom-DVE fast reciprocal requires a partition-0
                        # input) -- then fast-NR reciprocal, fold -lambda
                        # into the second half, broadcast across hd rows on
                        # Pool, then one mul on each of DVE/Pool and the
                        # final add on DVE.
                        o12 = cpool.tile([HD, 1024], F32, tag="o12",
                                         name=f"o12_{qc}_{h}")
                        rsum = cpool.tile([1, 1024], F32, tag="rsum",
                                          name=f"rsum_{qc}_{h}")
                        nc.vector.tensor_copy(rsum[:, 0:512], o1[HD:HD + 1, :])
                        nc.vector.tensor_copy(o12[:, 0:512], o1[0:HD, :])
                        nc.vector.tensor_copy(rsum[:, 512:1024], o2[HD:HD + 1, :])
                        nc.vector.tensor_copy(o12[:, 512:1024], o2[0:HD, :])
                        rr = rpool.tile([1, 1024], F32, tag="rr",
                                        name=f"rr_{qc}_{h}")
                        nc.vector.reciprocal_approx_fast(
                            out=rr[:], in_=rsum[:]
                        )
                        nc.vector.tensor_scalar_mul(
                            rr[:, 512:1024], rr[:, 512:1024], negl
                        )
                        rep = rpool.tile([HD, 1024], F32, tag="rep",
                                         name=f"rep_{qc}_{h}")
                        nc.gpsimd.partition_broadcast(rep[:], rr[:])
                        m1 = rpool.tile([HD, 512], F32, tag="m1",
                                        name=f"m1_{qc}_{h}")
                        nc.vector.tensor_mul(m1[:], o12[:, 0:512], rep[:, 0:512])
                        m2 = rpool.tile([HD, 512], F32, tag="m2",
                                        name=f"m2_{qc}_{h}")
                        nc.gpsimd.tensor_mul(
                            m2[:], o12[:, 512:1024], rep[:, 512:1024]
                        )
                        nc.vector.tensor_add(
                            o_t[h // 2][
                                (h % 2) * 64:(h % 2) * 64 + 64,
                                qc * 512:(qc + 1) * 512,
                            ],
                            m1[:],
                            m2[:],
                        )
                        # the previous chunk's projection, two column
                        # tiles per head so psum evacuations never bunch up
                        if qc > 0:
                            emit_proj_pair(qc - 1, h)
                for pair in range(4):
                    emit_proj_pair(QC - 1, pair)

              # ---- phase C: sumsq reduction ----
              with tc.tile_pool(name="psS", bufs=2, space="PSUM") as psS:
                for c4 in range(QC):
                    ssp = psS.tile([1, 512], F32, tag="ssqp", name=f"ssp_{c4}")
                    for t in range(2):
                        nc.tensor.matmul(
                            ssp[:],
                            lhsT=r(ones128[:]),
                            rhs=r(sq[t][:, c4 * 512:(c4 + 1) * 512]),
                            start=(t == 0),
                            stop=(t == 1),
                        )
                    nc.vector.tensor_copy(ssq_sb[:, c4 * 512:(c4 + 1) * 512], ssp[:])
                nc.sync.dma_start(ssq[:], ssq_sb[:])
    return nc


_CACHE = {}


def get_nc():
    if "nc" not in _CACHE:
        nc = bacc.Bacc(
            "TRN2", target_bir_lowering=False, debug=False, enable_asserts=False
        )
        build_program(nc)
        nc.compile()
        nc.m = get_hw_module(nc.m)
        _CACHE["nc"] = nc
    return _CACHE["nc"]


def make_in_maps(x, qkv_w, proj_w, lambda_q1, lambda_k1, lambda_q2, lambda_k2):
    x = np.asarray(x, np.float32)
    qkv_w = np.asarray(qkv_w, np.float32)
    proj_w = np.asarray(proj_w, np.float32)
    lamv = np.concatenate(
        [np.asarray(a, np.float32) for a in (lambda_q1, lambda_k1, lambda_q2, lambda_k2)]
    )[None, :]
    in_maps = []
    for core in range(8):
        b, hg = core // 4, core % 4
        h0 = hg * HPC
        rows = []
        for h in range(h0, h0 + HPC):
            rows.append(qkv_w[0 * DIM + h * HD:0 * DIM + (h + 1) * HD])
            rows.append(qkv_w[1 * DIM + h * HD:1 * DIM + (h + 1) * HD])
        for h in range(h0, h0 + HPC):
            rows.append(qkv_w[2 * DIM + h * HD:2 * DIM + (h + 1) * HD])
            rows.append(qkv_w[3 * DIM + h * HD:3 * DIM + (h + 1) * HD])
        wqk_tp = np.concatenate(rows, 0).T  # [DIM, 1024]
        wqk_np = np.ascontiguousarray(
            wqk_tp.reshape(CT, 128, 8 * 128).transpose(1, 0, 2).reshape(128, -1)
        )
        wv_tp = np.concatenate(
            [qkv_w[4 * DIM + h * HD:4 * DIM + (h + 1) * HD] for h in range(h0, h0 + HPC)],
            0,
        ).T  # [DIM, CH]
        wv_np = np.ascontiguousarray(
            wv_tp.reshape(CT, 128, CH).transpose(1, 0, 2).reshape(128, -1)
        )
        wp_np = np.ascontiguousarray(proj_w[:, h0 * HD:(h0 + HPC) * HD].T)
        in_maps.append(
            {
                "xt": np.ascontiguousarray(x[b].T),
                "wqk": wqk_np,
                "wv": wv_np,
                "wp": wp_np,
                "lam": np.ascontiguousarray(lamv),
            }
        )
    return in_maps


def combine(results, proj_b):
    proj_b = np.asarray(proj_b, np.float32)
    y = np.empty((B, N, DIM), np.float32)
    for b in range(B):
        acc = np.zeros((DIM, N), np.float64)
        sq = np.zeros(N, np.float64)
        for g in range(4):
            rr = results[b * 4 + g]
            acc += rr["out"].astype(np.float64)
            sq += rr["ssq"][0].astype(np.float64)
        s = 0.2 / np.sqrt(sq / DIM + EPS)
        y[b] = (acc.T * s[:, None] + proj_b).astype(np.float32)
    return y


def kernel(x, qkv_w, proj_w, proj_b, lambda_q1, lambda_k1, lambda_q2, lambda_k2):
    nc = get_nc()
    in_maps = make_in_maps(
        x, qkv_w, proj_w, lambda_q1, lambda_k1, lambda_q2, lambda_k2
    )
    res = bass_utils.run_bass_kernel_spmd(nc, in_maps, core_ids=list(range(8)))
    return combine(res.results, proj_b)

